# revision 25
# baseline (speedup 1.0000x reference)
"""Trainium2 Bass kernel for ChunkMessagePassing (gnn_message_passing).

Problem: B=2, N=4096, D=512, 3 rounds of causal windowed (W=8) message
passing. Per round:
    A = h @ w1_top ; Bv = h @ w1_bot + b1       (first MLP layer, factored)
    S[i] = sum_{k=0..8, valid} gelu(A[i] + Bv[i-k])
    hidden = gelu(h @ u1t + S @ Wc + ub1')      (Wc = (w2/9) @ u1b, host-folded;
                                                 b2 folded into ub1 on host)
    new_h = h + hidden @ u2 + ub2 ; h = LN(new_h)

Sharding: 8 cores = B(2) x N-quarters(4). Each core owns 1024 tokens plus a
24-token left halo (3 rounds x window 8) computed redundantly -> zero
cross-core communication. Sequence-start cores get a b1-filled margin plus a
data-driven edge fixup so all 8 cores run one SPMD program.

Layout: D on partitions (4 tiles of 128), tokens on the free axis. All
activations and weights fp16 (PSUM f32). Engine balance:
  - PE: 5 DxD matmuls/token/round (agg matmul folded into Wc on host) plus
    ones-matmul LN stats; software-pipelined emission (stage-1 of chunk c+1
    issues before stage-4/5 of chunk c, LN stats at lag 2) keeps the PE
    stream fed so the p-state ramp stays at full clock.
  - Pool (gpsimd): A/Bv PSUM evacuations + mub, off the ACT engine.
  - ACT: window gelu (bias-free; b1 rides the Bv evac), G gelu, x2 square.
  - DVE: fused window tap adds (2 instructions per half-chunk via strided
    APs with a k-reversed tmp layout), tap-sum tree, residual/LN chain.
  - LN: ones-matmul stats, rsqrt via f32 bit-trick + 1 Newton step; only
    gelu-set ACT functions are used so there is no table thrashing.
"""

import numpy as np

import concourse.bacc as bacc
import concourse.mybir as mybir
from concourse.tile import TileContext
from concourse.bass_utils import run_bass_kernel_spmd
from concourse.ap import AP

f32 = mybir.dt.float32
fp16 = mybir.dt.float16
u32 = mybir.dt.uint32
u16 = mybir.dt.uint16
AF = mybir.ActivationFunctionType
ALU = mybir.AluOpType

B, N, D = 2, 4096, 512
N_ROUNDS = 3
W = 8
W9 = W + 1
NCORES = 8
NLOC = N // 4            # tokens owned per core
HALO = N_ROUNDS * W      # 24
T = NLOC + HALO          # 1048 local tokens incl. halo
DT = 4                   # number of 128-partition d tiles
P = 128
MARG = 8                 # margin on the left of Bv buffers (holds b1)
BVW = MARG + T + 2
CN = 352                 # max chunk width
HCN = CN // 2            # window-stage half-chunk width
CHUNKS = [(0, 352), (352, 352), (704, 344)]
# per-round chunking: round 0 splits the first chunk (faster pipeline fill),
# every round splits the last chunk (shorter round-boundary / drain chain)
CHUNKS_R = [
    [(0, 88), (88, 88), (176, 176), (352, 352), (704, 172), (876, 172)],
    [(0, 352), (352, 352), (704, 172), (876, 172)],
    [(0, 352), (352, 352), (704, 172), (876, 172)],
]
# fp16 rsqrt seed magic, pre-scaled by sqrt(D): seeds y ~ sqrt(D)/sqrt(v)
MAGIC16 = 0x59BB + (9 << 9)

POOL_EVAC = False        # gpsimd cannot access PSUM on TRN2
FUSED_TAPS = True        # 2 strided tap instructions instead of 9


def build_nc(ln_affine=False):
    nc = bacc.Bacc("TRN2")

    # ---- DRAM I/O (per-core data supplied via in_maps) ----
    h_in = nc.dram_tensor("h_in", [DT, P, T], fp16, kind="ExternalInput")
    w1t_d = nc.dram_tensor("w1t", [DT, P, D], fp16, kind="ExternalInput")
    w1b_d = nc.dram_tensor("w1b", [DT, P, D], fp16, kind="ExternalInput")
    u1t_d = nc.dram_tensor("u1t", [DT, P, D], fp16, kind="ExternalInput")
    wc_d = nc.dram_tensor("wc", [DT, P, D], fp16, kind="ExternalInput")
    u2_d = nc.dram_tensor("u2", [DT, P, D], fp16, kind="ExternalInput")
    b1h_d = nc.dram_tensor("b1h", [P, DT], f32, kind="ExternalInput")
    ub1_d = nc.dram_tensor("ub1", [P, DT], f32, kind="ExternalInput")
    ub2_d = nc.dram_tensor("ub2", [P, DT], f32, kind="ExternalInput")
    lng_d = nc.dram_tensor("lng", [P, DT], f32, kind="ExternalInput")
    lnb_d = nc.dram_tensor("lnb", [P, DT], f32, kind="ExternalInput")
    ea_d = nc.dram_tensor("edge_a", [P, W], fp16, kind="ExternalInput")
    es_d = nc.dram_tensor("edge_s", [P, W], fp16, kind="ExternalInput")
    hm_d = nc.dram_tensor("hmask", [P, HALO], fp16, kind="ExternalInput")
    out_d = nc.dram_tensor("out", [DT, P, NLOC], fp16, kind="ExternalOutput")

    with nc.allow_low_precision("fp16 compute validated against reference"), \
            TileContext(nc) as tc:
        with (
            tc.tile_pool(name="const", bufs=1) as cp,
            tc.tile_pool(name="acts", bufs=1) as ap,
            tc.tile_pool(name="win", bufs=3) as wpw,
            tc.tile_pool(name="x2p", bufs=2) as wpx,
            tc.tile_pool(name="lns", bufs=8) as wps,
            tc.tile_pool(name="edg", bufs=2) as wpe,
            tc.tile_pool(name="psab", bufs=2, space="PSUM") as psab,
            tc.tile_pool(name="ps", bufs=2, space="PSUM") as ps,
            tc.tile_pool(name="psr", bufs=2, space="PSUM") as psr,
        ):
            # ---- constants into SBUF ----
            w1t = cp.tile([P, DT * D], fp16, tag="w1t")
            w1b = cp.tile([P, DT * D], fp16, tag="w1b")
            u1t = cp.tile([P, DT * D], fp16, tag="u1t")
            wc = cp.tile([P, DT * D], fp16, tag="wc")
            u2 = cp.tile([P, DT * D], fp16, tag="u2")
            b1h = cp.tile([P, DT], f32, tag="b1h")
            ub1 = cp.tile([P, DT], f32, tag="ub1")
            ub2 = cp.tile([P, DT], f32, tag="ub2")
            lng = cp.tile([P, DT], f32, tag="lng")
            lnb = cp.tile([P, DT], f32, tag="lnb")
            edge_a = cp.tile([P, W], fp16, tag="edge_a")
            edge_s = cp.tile([P, W], fp16, tag="edge_s")
            hmask = cp.tile([P, HALO], fp16, tag="hmask")
            # smalls first on the scalar queue (b1h gates the margins), then
            # nothing else on scalar so chunk-0 evacuations start promptly
            for t_sb, t_d in ((b1h, b1h_d), (ub1, ub1_d), (ub2, ub2_d),
                              (lng, lng_d), (lnb, lnb_d), (edge_a, ea_d),
                              (edge_s, es_d), (hmask, hm_d)):
                nc.scalar.dma_start(out=t_sb[:], in_=t_d[:])
            # weight order on sync matches first-use order; the h chunk for
            # tokens 352..704 rides between so stage-1 of chunk 2 is fed
            for t_sb, t_d in ((w1t, w1t_d), (w1b, w1b_d)):
                for kt in range(DT):
                    nc.sync.dma_start(out=t_sb[:, kt * D: (kt + 1) * D],
                                      in_=t_d[kt])

            ones_b = cp.tile([P, P], fp16, tag="ones_b")   # stats lhsT
            nc.vector.memset(ones_b[:], 1.0)
            mg = cp.tile([P, CN], u16, tag="mg")           # rsqrt seed magic
            nc.vector.memset(mg[:], MAGIC16)
            czero = cp.tile([P, 1], f32, tag="czero")
            nc.vector.memset(czero[:], 0.0)
            nc.const_aps.aps[(f32, 0.0)] = czero[:]
            # dummy gelu so the preamble loads the gelu act table once,
            # instead of a mid-stream 1.3us ACT_TABLE_LOAD stall
            warm = cp.tile([P, 1], fp16, tag="warm")
            nc.scalar.activation(warm[:], czero[:], AF.Gelu)

            # ---- activations (persistent, reused across rounds) ----
            h0 = ap.tile([P, DT * T], fp16, tag="h0")
            h1 = ap.tile([P, DT * T], fp16, tag="h1")
            # A and Bv share one tensor so the paired PSUM evacuation can
            # write both with a single strided instruction
            ABt = ap.tile([P, DT * T + DT * BVW], fp16, tag="ABt")
            AOFF = 0
            BVOFF = DT * T
            G = ap.tile([P, DT * T], fp16, tag="G")
            Bvo = ap.tile([P, DT * BVW], fp16, tag="Bvo")
            S = ap.tile([P, DT * T], fp16, tag="S")

            # round-1 input: chunks 0,2 on gpsimd (early), chunk 1 on sync
            # (rides between the w1 and u1 weight groups)
            for li, (c0, cn) in enumerate(CHUNKS):
                q = nc.gpsimd if li != 1 else nc.sync
                for dt in range(DT):
                    q.dma_start(out=h0[:, dt * T + c0: dt * T + c0 + cn],
                                in_=h_in[dt, :, c0: c0 + cn])
            for t_sb, t_d in ((u1t, u1t_d), (wc, wc_d), (u2, u2_d)):
                for kt in range(DT):
                    nc.sync.dma_start(out=t_sb[:, kt * D: (kt + 1) * D],
                                      in_=t_d[kt])

            # margins: b1/2 per dt (both A and Bv carry half of b1, so
            # invalid window taps see gelu(A + b1/2 + b1/2) = gelu(A + b1))
            for dt in range(DT):
                nc.vector.tensor_copy(
                    ABt[:, BVOFF + dt * BVW: BVOFF + dt * BVW + MARG],
                    b1h[:, dt: dt + 1].to_broadcast([P, MARG]))
                nc.vector.tensor_copy(
                    Bvo[:, dt * BVW: dt * BVW + MARG + 1],
                    b1h[:, dt: dt + 1].to_broadcast([P, MARG + 1]))

            def hsl(h, dt, c0, n):
                return h[:, dt * T + c0: dt * T + c0 + n]

            def asl(dt, c0, n):
                return ABt[:, AOFF + dt * T + c0: AOFF + dt * T + c0 + n]

            def wtile(w, kt, dt):
                return w[:, kt * D + dt * P: kt * D + dt * P + P]

            def d4(tile, c0, cn):       # [P, 4dt, cn] view of a [P, DT*T] tile
                return tile[:].rearrange("p (d t) -> p d t", d=DT)[:, :, c0:c0 + cn]

            a_part = ABt[:][:, AOFF: AOFF + DT * T]
            a4 = a_part.rearrange("p (d t) -> p d t", d=DT)
            bv_flat = ABt[:][:, BVOFF: BVOFF + DT * BVW]
            bv4 = bv_flat.rearrange("p (d v) -> p d v", d=DT)
            bvo4 = Bvo[:].rearrange("p (d v) -> p d v", d=DT)
            bvo_flat = Bvo[:]

            def strided4(base, off, dims):
                # raw AP: dims = [(stride, num), ...] free dims (<=3)
                return AP(tensor=base.tensor, offset=base.offset + off,
                          ap=[list(base.ap[0])] + [list(d) for d in dims])

            hbufs = [h0, h1]

            # ---------- per-chunk emitters ----------
            def emit_s1(r, ci, c0, cn):
                hin = hbufs[r % 2]
                abf = ABt[:]
                for dt in range(DT):
                    pab = psab.tile([P, 1024], f32, tag="pab")
                    for kt in range(DT):
                        nc.tensor.matmul(pab[:, :cn], wtile(w1t, kt, dt),
                                         hsl(hin, kt, c0, cn),
                                         start=(kt == 0), stop=(kt == DT - 1))
                    for kt in range(DT):
                        nc.tensor.matmul(pab[:, 512: 512 + cn],
                                         wtile(w1b, kt, dt),
                                         hsl(hin, kt, c0, cn),
                                         start=(kt == 0), stop=(kt == DT - 1))
                    # paired evacuation: A' = A + b1/2, Bv' = Bv + b1/2 in
                    # one strided ACT instruction across the 2 PSUM banks
                    psrc = pab[:]
                    src = AP(tensor=psrc.tensor, offset=psrc.offset,
                             ap=[list(psrc.ap[0]), [512, 2], [1, cn]])
                    a_at = AOFF + dt * T + c0
                    gap = (BVOFF + dt * BVW + MARG + c0) - a_at
                    dst = AP(tensor=abf.tensor, offset=abf.offset + a_at,
                             ap=[list(abf.ap[0]), [gap, 2], [1, cn]])
                    nc.scalar.activation(dst, src, AF.Identity,
                                         bias=b1h[:, dt: dt + 1])
                # odd-tap-aligned copy of Bv (SBUF->SBUF, all dt fused)
                nc.vector.tensor_copy(
                    bvo4[:, :, MARG + 1 + c0: MARG + 1 + c0 + cn],
                    bv4[:, :, MARG + c0: MARG + c0 + cn])

            def emit_win(r, ci, c0, cn):
                hn = cn // 2
                for hb in (0, hn):
                    hc0 = c0 + hb
                    tmph = wpw.tile([P, DT * W9 * HCN], fp16, tag="tmp")
                    gh = wpw.tile([P, DT * W9 * HCN], fp16, tag="g")
                    tmp4 = tmph[:].rearrange("p (d k i) -> p d k i",
                                             d=DT, k=W9)
                    g4 = gh[:].rearrange("p (d k i) -> p d k i",
                                         d=DT, k=W9)
                    a4c = a4[:, :, hc0:hc0 + hn]
                    if FUSED_TAPS:
                        # tmp slot j holds tap k = 8-j (order-free: summed)
                        tb = tmph[:]
                        # evens j=0,2,4,6,8 <-> k=8,6,4,2,0 from Bv
                        out_e = strided4(tb, 0, [(W9 * HCN, DT),
                                                 (2 * HCN, 5), (1, hn)])
                        in_e = strided4(bv_flat, MARG + hc0 - 8,
                                        [(BVW, DT), (2, 5), (1, hn)])
                        a_b5 = a4c.unsqueeze(2).to_broadcast([P, DT, 5, hn])
                        nc.vector.tensor_tensor(out_e, a_b5, in_e, ALU.add)
                        # odds j=1,3,5,7 <-> k=7,5,3,1 from Bvo
                        out_o = strided4(tb, HCN, [(W9 * HCN, DT),
                                                   (2 * HCN, 4), (1, hn)])
                        in_o = strided4(bvo_flat, MARG + hc0 - 6,
                                        [(BVW, DT), (2, 4), (1, hn)])
                        a_b4 = a4c.unsqueeze(2).to_broadcast([P, DT, 4, hn])
                        nc.vector.tensor_tensor(out_o, a_b4, in_o, ALU.add)
                    else:
                        for k in range(W9):
                            if k % 2 == 0:
                                bsl = bv4[:, :, MARG + hc0 - k:
                                          MARG + hc0 - k + hn]
                            else:
                                bsl = bvo4[:, :, MARG + 1 + hc0 - k:
                                           MARG + 1 + hc0 - k + hn]
                            nc.vector.tensor_tensor(tmp4[:, :, k, 0:hn], a4c,
                                                    bsl, ALU.add)
                    nc.scalar.activation(g4[:, :, :, 0:hn],
                                         tmp4[:, :, :, 0:hn], AF.Gelu)
                    nc.vector.tensor_tensor(tmp4[:, :, 0:4, 0:hn],
                                            g4[:, :, 0:4, 0:hn],
                                            g4[:, :, 4:8, 0:hn], ALU.add)
                    nc.vector.tensor_tensor(tmp4[:, :, 0:2, 0:hn],
                                            tmp4[:, :, 0:2, 0:hn],
                                            tmp4[:, :, 2:4, 0:hn], ALU.add)
                    nc.vector.tensor_tensor(tmp4[:, :, 0, 0:hn],
                                            tmp4[:, :, 0, 0:hn],
                                            tmp4[:, :, 1, 0:hn], ALU.add)
                    nc.vector.tensor_tensor(d4(S, hc0, hn),
                                            tmp4[:, :, 0, 0:hn],
                                            g4[:, :, 8, 0:hn], ALU.add)

                # ---- edge fixup (chunk 0 only; no-op off sequence starts)
                if ci == 0:
                    ga8 = wpe.tile([P, DT * W], fp16, tag="ga8")
                    for dt in range(DT):
                        nc.scalar.activation(
                            ga8[:, dt * W: dt * W + W],
                            asl(dt, HALO, W),
                            AF.Gelu, bias=b1h[:, dt: dt + 1])
                    ga84 = ga8[:].rearrange("p (d w) -> p d w", d=DT)
                    s4e = d4(S, HALO, W)
                    ea_b = edge_a[:].unsqueeze(1).to_broadcast([P, DT, W])
                    es_b = edge_s[:].unsqueeze(1).to_broadcast([P, DT, W])
                    nc.vector.tensor_tensor(ga84, ga84, ea_b, ALU.mult)
                    nc.vector.tensor_tensor(s4e, s4e, ga84, ALU.subtract)
                    nc.vector.tensor_tensor(s4e, s4e, es_b, ALU.mult)

            def emit_r1(r, ci, c0, cn):
                hin = hbufs[r % 2]
                hout = hbufs[(r + 1) % 2]
                # ---- stage 4: U = u1t.T@h + wc.T@S ; G = gelu(U + ub1')
                for dt in range(DT):
                    pu = ps.tile([P, 512], f32, tag="pmm")
                    for kt in range(DT):
                        nc.tensor.matmul(pu[:, :cn], wtile(u1t, kt, dt),
                                         hsl(hin, kt, c0, cn),
                                         start=(kt == 0), stop=False)
                    for kt in range(DT):
                        nc.tensor.matmul(pu[:, :cn], wtile(wc, kt, dt),
                                         hsl(S, kt, c0, cn),
                                         start=False, stop=(kt == DT - 1))
                    nc.scalar.activation(hsl(G, dt, c0, cn), pu[:, :cn],
                                         AF.Gelu, bias=ub1[:, dt: dt + 1])
                # ---- stage 5: hout = (u2.T@G + ub2) + h  (residual in evac)
                for dt in range(DT):
                    pv = ps.tile([P, 512], f32, tag="pmm")
                    for kt in range(DT):
                        nc.tensor.matmul(pv[:, :cn], wtile(u2, kt, dt),
                                         hsl(G, kt, c0, cn),
                                         start=(kt == 0), stop=(kt == DT - 1))
                    nc.vector.scalar_tensor_tensor(
                        hsl(hout, dt, c0, cn), pv[:, :cn],
                        ub2[:, dt: dt + 1], hsl(hin, dt, c0, cn),
                        ALU.add, ALU.add)
                # x2 = hout^2 on ACT (square is in the gelu table set)
                x2t = wpx.tile([P, DT * CN], fp16, tag="x2")
                x24 = x2t[:].rearrange("p (d i) -> p d i", d=DT)[:, :, 0:cn]
                hout4 = d4(hout, c0, cn)
                nc.scalar.activation(x24, hout4, AF.Square)
                return x2t

            def emit_r2(r, ci, c0, cn, x2t):
                hin = hbufs[r % 2]
                hout = hbufs[(r + 1) % 2]
                hout4 = d4(hout, c0, cn)
                x24 = x2t[:].rearrange("p (d i) -> p d i", d=DT)[:, :, 0:cn]
                # ---- LN stats via ones-matmul
                pr0 = psr.tile([P, 512], f32, tag="prow")
                pr1 = psr.tile([P, 512], f32, tag="prow")
                for kt in range(DT):
                    nc.tensor.matmul(pr0[:, :cn], ones_b[:],
                                     hsl(hout, kt, c0, cn),
                                     start=(kt == 0), stop=(kt == DT - 1))
                for kt in range(DT):
                    nc.tensor.matmul(pr1[:, :cn], ones_b[:],
                                     x2t[:, kt * CN: kt * CN + cn],
                                     start=(kt == 0), stop=(kt == DT - 1))
                mub = wps.tile([P, CN], fp16, tag="mub")
                qq = wps.tile([P, CN], fp16, tag="qq")
                vt = wps.tile([P, CN], fp16, tag="vt")
                y0 = wps.tile([P, CN], fp16, tag="y0")
                tt = wps.tile([P, CN], fp16, tag="tt")
                rsth = wps.tile([P, CN], fp16, tag="rsth")
                # mub = -pr0/D (ACT, parallel to the rsqrt chain)
                nc.scalar.activation(mub[:, :cn], pr0[:, :cn], AF.Copy,
                                     scale=-1.0 / D)
                # vt = D*var = pr1 - pr0^2/D; rst = rsqrt(var) computed as
                # sqrt(D)*rsqrt(vt) via a sqrt(D)-scaled seed magic and a
                # Newton step with 0.5/D in place of 0.5
                nc.vector.tensor_tensor(qq[:, :cn], mub[:, :cn],
                                        mub[:, :cn], ALU.mult)
                nc.vector.scalar_tensor_tensor(vt[:, :cn], qq[:, :cn],
                                               -float(D), pr1[:, :cn],
                                               ALU.mult, ALU.add)
                nc.vector.tensor_scalar(y0[:, :cn].bitcast(u16),
                                        vt[:, :cn].bitcast(u16), 1, None,
                                        ALU.logical_shift_right)
                nc.vector.tensor_tensor(y0[:, :cn].bitcast(u16),
                                        mg[:, :cn], y0[:, :cn].bitcast(u16),
                                        ALU.subtract)
                nc.vector.tensor_tensor(tt[:, :cn], y0[:, :cn], y0[:, :cn],
                                        ALU.mult)
                nc.vector.scalar_tensor_tensor(tt[:, :cn], tt[:, :cn],
                                               -0.5 / D, vt[:, :cn],
                                               ALU.mult, ALU.mult)
                nc.vector.scalar_tensor_tensor(rsth[:, :cn], tt[:, :cn], 1.5,
                                               y0[:, :cn], ALU.add, ALU.mult)
                # xm = hout - mu ; hout = xm * rst (final mult per dt so
                # next-round matmuls start as soon as each d-tile lands)
                mub_b = mub[:, :cn].unsqueeze(1).to_broadcast([P, DT, cn])
                nc.vector.tensor_tensor(x24, hout4, mub_b, ALU.add)
                for dt in range(DT):
                    nc.vector.tensor_tensor(
                        hsl(hout, dt, c0, cn),
                        x2t[:, dt * CN: dt * CN + cn], rsth[:, :cn],
                        ALU.mult)
                if ln_affine:
                    for dt in range(DT):
                        nc.vector.tensor_scalar(
                            hsl(hout, dt, c0, cn), hsl(hout, dt, c0, cn),
                            lng[:, dt: dt + 1], lnb[:, dt: dt + 1],
                            ALU.mult, ALU.add)

                # zero pad margin on sequence-start cores (chunk 0)
                if ci == 0 and r < N_ROUNDS - 1:
                    hm_b = hmask[:].unsqueeze(1).to_broadcast([P, DT, HALO])
                    nc.gpsimd.tensor_tensor(d4(hout, 0, HALO),
                                            d4(hout, 0, HALO), hm_b,
                                            ALU.mult)
                # stream the final round's output per chunk (overlaps drain)
                if r == N_ROUNDS - 1:
                    qs = [nc.sync, nc.gpsimd]
                    lo = max(c0, HALO)
                    hi = c0 + cn
                    for dt in range(DT):
                        qs[(dt + ci) % 2].dma_start(
                            out=out_d[dt, :, lo - HALO: hi - HALO],
                            in_=hsl(hout, dt, lo, hi - lo))

            # ---------- software-pipelined emission ----------
            # per iteration: S1(c) | R1(c-1) | W(c) | R2(c-2)
            pend1 = []   # chunks awaiting R1
            pend2 = []   # chunks awaiting R2 (carry x2t from R1)
            for r in range(N_ROUNDS):
                for ci, (c0, cn) in enumerate(CHUNKS_R[r]):
                    emit_s1(r, ci, c0, cn)
                    if pend1:
                        args = pend1.pop(0)
                        x2t = emit_r1(*args)
                        pend2.append((*args, x2t))
                    emit_win(r, ci, c0, cn)
                    if len(pend2) >= 2:
                        emit_r2(*pend2.pop(0))
                    pend1.append((r, ci, c0, cn))
            while pend1:
                args = pend1.pop(0)
                x2t = emit_r1(*args)
                pend2.append((*args, x2t))
                if pend2:
                    emit_r2(*pend2.pop(0))
            while pend2:
                emit_r2(*pend2.pop(0))

    nc.finalize()
    return nc


_NC_CACHE = {}


def _get_nc(ln_affine=False):
    key = ("nc", ln_affine)
    if key not in _NC_CACHE:
        _NC_CACHE[key] = build_nc(ln_affine)
    return _NC_CACHE[key]


def _prep_inputs(chunk_summaries, msg_w1, msg_b1, msg_w2, msg_b2,
                 upd_w1, upd_b1, upd_w2, upd_b2, ln_g, ln_b):
    h = np.asarray(chunk_summaries, np.float32)          # (B, N, D)
    w1 = np.asarray(msg_w1, np.float32)                  # (2D, D)
    w2 = np.asarray(msg_w2, np.float32)                  # (D, D)
    u1 = np.asarray(upd_w1, np.float32)
    u2 = np.asarray(upd_w2, np.float32)
    lng = np.asarray(ln_g, np.float32)
    lnb = np.asarray(ln_b, np.float32)
    ln_affine = not (np.all(lng == 1.0) and np.all(lnb == 0.0))

    # fold msg_b2 into the update-MLP hidden bias: agg@u1b + (b2@u1b + ub1)
    ub1f = (np.asarray(upd_b1, np.float64)
            + np.asarray(msg_b2, np.float64) @ np.asarray(u1[D:], np.float64)
            ).astype(np.float32)
    # fold the msg second layer into the update first layer:
    # agg@u1b = (S/9 @ w2)@u1b = S @ ((w2/9) @ u1b)
    wcomb = ((np.asarray(w2, np.float64) / 9.0)
             @ np.asarray(u1[D:], np.float64)).astype(np.float32)

    def pack_w(w):
        return np.ascontiguousarray(w.reshape(DT, P, D).astype(np.float16))

    def pack_b(b):
        return np.ascontiguousarray(np.asarray(b, np.float32).reshape(DT, P).T)

    common = {
        "w1t": pack_w(w1[:D]),
        "w1b": pack_w(w1[D:]),
        "u1t": pack_w(u1[:D]),
        "wc": pack_w(wcomb),
        "u2": pack_w(u2),
        "b1h": pack_b(np.asarray(msg_b1, np.float64) * 0.5),
        "ub1": pack_b(ub1f),
        "ub2": pack_b(upd_b2),
        "lng": pack_b(lng),
        "lnb": pack_b(lnb),
    }

    i8 = np.arange(W, dtype=np.float32)
    ea_edge = np.broadcast_to((W - i8), (P, W)).astype(np.float16)
    es_edge = np.broadcast_to((9.0 / (i8 + 1.0)), (P, W)).astype(np.float16)
    ea_mid = np.zeros((P, W), np.float16)
    es_mid = np.ones((P, W), np.float16)
    hm_edge = np.zeros((P, HALO), np.float16)
    hm_mid = np.ones((P, HALO), np.float16)

    in_maps = []
    for core in range(NCORES):
        b = core // 4
        q = core % 4
        n0 = q * NLOC
        if q == 0:
            loc = np.zeros((T, D), np.float32)
            loc[HALO:] = h[b, :NLOC]
            ea, es, hm = ea_edge, es_edge, hm_edge
        else:
            loc = h[b, n0 - HALO: n0 + NLOC]
            ea, es, hm = ea_mid, es_mid, hm_mid
        hloc = np.ascontiguousarray(
            loc.T.reshape(DT, P, T).astype(np.float16))
        m = dict(common)
        m["h_in"] = hloc
        m["edge_a"] = ea
        m["edge_s"] = es
        m["hmask"] = hm
        in_maps.append(m)
    return in_maps, ln_affine


def kernel(**inputs) -> np.ndarray:
    in_maps, ln_affine = _prep_inputs(**inputs)
    nc = _get_nc(ln_affine)
    res = run_bass_kernel_spmd(nc, in_maps, list(range(NCORES)))
    out = np.empty((B, N, D), np.float32)
    for core in range(NCORES):
        b = core // 4
        q = core % 4
        o = np.asarray(res.results[core]["out"]).astype(np.float32)
        out[b, q * NLOC:(q + 1) * NLOC] = o.reshape(D, NLOC).T
    return out


# revision 26
# speedup vs baseline: 1.0440x; 1.0440x over previous
"""Trainium2 Bass kernel for ChunkMessagePassing (gnn_message_passing).

Problem: B=2, N=4096, D=512, 3 rounds of causal windowed (W=8) message
passing. Per round:
    A = h @ w1_top ; Bv = h @ w1_bot + b1       (first MLP layer, factored)
    S[i] = sum_{k=0..8, valid} gelu(A[i] + Bv[i-k])
    hidden = gelu(h @ u1t + S @ Wc + ub1')      (Wc = (w2/9) @ u1b, host-folded;
                                                 b2 folded into ub1 on host)
    new_h = h + hidden @ u2 + ub2 ; h = LN(new_h)

Sharding: 8 cores = B(2) x N-quarters(4). Each core owns 1024 tokens plus a
24-token left halo (3 rounds x window 8) computed redundantly -> zero
cross-core communication. Sequence-start cores get a b1-filled margin plus a
data-driven edge fixup so all 8 cores run one SPMD program.

Layout: D on partitions (4 tiles of 128), tokens on the free axis. All
activations and weights fp16 (PSUM f32). Engine balance:
  - PE: 5 DxD matmuls/token/round (agg matmul folded into Wc on host) plus
    ones-matmul LN stats; software-pipelined emission (stage-1 of chunk c+1
    issues before stage-4/5 of chunk c, LN stats at lag 2) keeps the PE
    stream fed so the p-state ramp stays at full clock.
  - Pool (gpsimd): A/Bv PSUM evacuations + mub, off the ACT engine.
  - ACT: window gelu (bias-free; b1 rides the Bv evac), G gelu, x2 square.
  - DVE: fused window tap adds (2 instructions per half-chunk via strided
    APs with a k-reversed tmp layout), tap-sum tree, residual/LN chain.
  - LN: ones-matmul stats, rsqrt via f32 bit-trick + 1 Newton step; only
    gelu-set ACT functions are used so there is no table thrashing.
"""

import numpy as np

import concourse.bacc as bacc
import concourse.mybir as mybir
from concourse.tile import TileContext
from concourse.bass_utils import run_bass_kernel_spmd
from concourse.ap import AP

f32 = mybir.dt.float32
fp16 = mybir.dt.float16
u32 = mybir.dt.uint32
u16 = mybir.dt.uint16
AF = mybir.ActivationFunctionType
ALU = mybir.AluOpType

B, N, D = 2, 4096, 512
N_ROUNDS = 3
W = 8
W9 = W + 1
NCORES = 8
NLOC = N // 4            # tokens owned per core
HALO = N_ROUNDS * W      # 24
T = NLOC + HALO          # 1048 local tokens incl. halo
DT = 4                   # number of 128-partition d tiles
P = 128
MARG = 8                 # margin on the left of Bv buffers (holds b1)
BVW = MARG + T + 2
CN = 352                 # max chunk width
HCN = CN // 2            # window-stage half-chunk width
CHUNKS = [(0, 352), (352, 352), (704, 344)]
# per-round chunking: round 0 splits the first chunk (faster pipeline fill),
# every round splits the last chunk (shorter round-boundary / drain chain)
CHUNKS_R = [
    [(0, 88), (88, 88), (176, 176), (352, 352), (704, 172), (876, 172)],
    [(0, 352), (352, 352), (704, 172), (876, 172)],
    [(0, 352), (352, 352), (704, 172), (876, 172)],
]
# fp16 rsqrt seed magic, pre-scaled by sqrt(D): seeds y ~ sqrt(D)/sqrt(v)
MAGIC16 = 0x59BB + (9 << 9)

POOL_EVAC = False        # gpsimd cannot access PSUM on TRN2
FUSED_TAPS = True        # 2 strided tap instructions instead of 9


def build_nc(ln_affine=False):
    nc = bacc.Bacc("TRN2")

    # ---- DRAM I/O (per-core data supplied via in_maps) ----
    h_in = nc.dram_tensor("h_in", [DT, P, T], fp16, kind="ExternalInput")
    w1t_d = nc.dram_tensor("w1t", [DT, P, D], fp16, kind="ExternalInput")
    w1b_d = nc.dram_tensor("w1b", [DT, P, D], fp16, kind="ExternalInput")
    u1t_d = nc.dram_tensor("u1t", [DT, P, D], fp16, kind="ExternalInput")
    wc_d = nc.dram_tensor("wc", [DT, P, D], fp16, kind="ExternalInput")
    u2_d = nc.dram_tensor("u2", [DT, P, D], fp16, kind="ExternalInput")
    b1h_d = nc.dram_tensor("b1h", [P, DT], f32, kind="ExternalInput")
    ub1_d = nc.dram_tensor("ub1", [P, DT], f32, kind="ExternalInput")
    ub2_d = nc.dram_tensor("ub2", [P, DT], f32, kind="ExternalInput")
    lng_d = nc.dram_tensor("lng", [P, DT], f32, kind="ExternalInput")
    lnb_d = nc.dram_tensor("lnb", [P, DT], f32, kind="ExternalInput")
    ea_d = nc.dram_tensor("edge_a", [P, W], fp16, kind="ExternalInput")
    es_d = nc.dram_tensor("edge_s", [P, W], fp16, kind="ExternalInput")
    hm_d = nc.dram_tensor("hmask", [P, HALO], fp16, kind="ExternalInput")
    out_d = nc.dram_tensor("out", [DT, P, NLOC], fp16, kind="ExternalOutput")

    with nc.allow_low_precision("fp16 compute validated against reference"), \
            TileContext(nc) as tc:
        with (
            tc.tile_pool(name="const", bufs=1) as cp,
            tc.tile_pool(name="acts", bufs=1) as ap,
            tc.tile_pool(name="win", bufs=3) as wpw,
            tc.tile_pool(name="x2p", bufs=2) as wpx,
            tc.tile_pool(name="lns", bufs=8) as wps,
            tc.tile_pool(name="edg", bufs=2) as wpe,
            tc.tile_pool(name="psab", bufs=2, space="PSUM") as psab,
            tc.tile_pool(name="ps", bufs=2, space="PSUM") as ps,
            tc.tile_pool(name="psr", bufs=2, space="PSUM") as psr,
        ):
            # ---- constants into SBUF ----
            w1t = cp.tile([P, DT * D], fp16, tag="w1t")
            w1b = cp.tile([P, DT * D], fp16, tag="w1b")
            u1t = cp.tile([P, DT * D], fp16, tag="u1t")
            wc = cp.tile([P, DT * D], fp16, tag="wc")
            u2 = cp.tile([P, DT * D], fp16, tag="u2")
            b1h = cp.tile([P, DT], f32, tag="b1h")
            ub1 = cp.tile([P, DT], f32, tag="ub1")
            ub2 = cp.tile([P, DT], f32, tag="ub2")
            lng = cp.tile([P, DT], f32, tag="lng")
            lnb = cp.tile([P, DT], f32, tag="lnb")
            edge_a = cp.tile([P, W], fp16, tag="edge_a")
            edge_s = cp.tile([P, W], fp16, tag="edge_s")
            hmask = cp.tile([P, HALO], fp16, tag="hmask")
            # smalls first on the scalar queue (b1h gates the margins), then
            # nothing else on scalar so chunk-0 evacuations start promptly
            for t_sb, t_d in ((b1h, b1h_d), (ub1, ub1_d), (ub2, ub2_d),
                              (lng, lng_d), (lnb, lnb_d), (edge_a, ea_d),
                              (edge_s, es_d), (hmask, hm_d)):
                nc.scalar.dma_start(out=t_sb[:], in_=t_d[:])
            # weight order on sync matches first-use order; the h chunk for
            # tokens 352..704 rides between so stage-1 of chunk 2 is fed
            for t_sb, t_d in ((w1t, w1t_d), (w1b, w1b_d)):
                for kt in range(DT):
                    nc.sync.dma_start(out=t_sb[:, kt * D: (kt + 1) * D],
                                      in_=t_d[kt])

            ones_b = cp.tile([P, P], fp16, tag="ones_b")   # stats lhsT
            nc.vector.memset(ones_b[:], 1.0)
            mg = cp.tile([P, CN], u16, tag="mg")           # rsqrt seed magic
            nc.vector.memset(mg[:], MAGIC16)
            czero = cp.tile([P, 1], f32, tag="czero")
            nc.vector.memset(czero[:], 0.0)
            nc.const_aps.aps[(f32, 0.0)] = czero[:]
            # dummy gelu so the preamble loads the gelu act table once,
            # instead of a mid-stream 1.3us ACT_TABLE_LOAD stall
            warm = cp.tile([P, 1], fp16, tag="warm")
            nc.scalar.activation(warm[:], czero[:], AF.Gelu)

            # ---- activations (persistent, reused across rounds) ----
            h0 = ap.tile([P, DT * T], fp16, tag="h0")
            h1 = ap.tile([P, DT * T], fp16, tag="h1")
            # A and Bv share one tensor so the paired PSUM evacuation can
            # write both with a single strided instruction
            ABt = ap.tile([P, DT * T + DT * BVW], fp16, tag="ABt")
            AOFF = 0
            BVOFF = DT * T
            G = ap.tile([P, DT * T], fp16, tag="G")
            Bvo = ap.tile([P, DT * BVW], fp16, tag="Bvo")
            S = ap.tile([P, DT * T], fp16, tag="S")

            # round-1 input: chunks 0,2 on gpsimd (early), chunk 1 on sync
            # (rides between the w1 and u1 weight groups)
            for li, (c0, cn) in enumerate(CHUNKS):
                q = nc.gpsimd if li != 1 else nc.sync
                for dt in range(DT):
                    q.dma_start(out=h0[:, dt * T + c0: dt * T + c0 + cn],
                                in_=h_in[dt, :, c0: c0 + cn])
            for t_sb, t_d in ((u1t, u1t_d), (wc, wc_d), (u2, u2_d)):
                for kt in range(DT):
                    nc.sync.dma_start(out=t_sb[:, kt * D: (kt + 1) * D],
                                      in_=t_d[kt])

            # margins: b1/2 per dt (both A and Bv carry half of b1, so
            # invalid window taps see gelu(A + b1/2 + b1/2) = gelu(A + b1))
            for dt in range(DT):
                nc.vector.tensor_copy(
                    ABt[:, BVOFF + dt * BVW: BVOFF + dt * BVW + MARG],
                    b1h[:, dt: dt + 1].to_broadcast([P, MARG]))
                nc.vector.tensor_copy(
                    Bvo[:, dt * BVW: dt * BVW + MARG + 1],
                    b1h[:, dt: dt + 1].to_broadcast([P, MARG + 1]))

            def hsl(h, dt, c0, n):
                return h[:, dt * T + c0: dt * T + c0 + n]

            def asl(dt, c0, n):
                return ABt[:, AOFF + dt * T + c0: AOFF + dt * T + c0 + n]

            def wtile(w, kt, dt):
                return w[:, kt * D + dt * P: kt * D + dt * P + P]

            def d4(tile, c0, cn):       # [P, 4dt, cn] view of a [P, DT*T] tile
                return tile[:].rearrange("p (d t) -> p d t", d=DT)[:, :, c0:c0 + cn]

            a_part = ABt[:][:, AOFF: AOFF + DT * T]
            a4 = a_part.rearrange("p (d t) -> p d t", d=DT)
            bv_flat = ABt[:][:, BVOFF: BVOFF + DT * BVW]
            bv4 = bv_flat.rearrange("p (d v) -> p d v", d=DT)
            bvo4 = Bvo[:].rearrange("p (d v) -> p d v", d=DT)
            bvo_flat = Bvo[:]

            def strided4(base, off, dims):
                # raw AP: dims = [(stride, num), ...] free dims (<=3)
                return AP(tensor=base.tensor, offset=base.offset + off,
                          ap=[list(base.ap[0])] + [list(d) for d in dims])

            hbufs = [h0, h1]

            # ---------- per-chunk emitters ----------
            def emit_s1(r, ci, c0, cn):
                hin = hbufs[r % 2]
                abf = ABt[:]
                for dt in range(DT):
                    pab = psab.tile([P, 1024], f32, tag="pab")
                    for kt in range(DT):
                        nc.tensor.matmul(pab[:, :cn], wtile(w1t, kt, dt),
                                         hsl(hin, kt, c0, cn),
                                         start=(kt == 0), stop=(kt == DT - 1))
                    for kt in range(DT):
                        nc.tensor.matmul(pab[:, 512: 512 + cn],
                                         wtile(w1b, kt, dt),
                                         hsl(hin, kt, c0, cn),
                                         start=(kt == 0), stop=(kt == DT - 1))
                    # paired evacuation: A' = A + b1/2, Bv' = Bv + b1/2 in
                    # one strided ACT instruction across the 2 PSUM banks
                    psrc = pab[:]
                    src = AP(tensor=psrc.tensor, offset=psrc.offset,
                             ap=[list(psrc.ap[0]), [512, 2], [1, cn]])
                    a_at = AOFF + dt * T + c0
                    gap = (BVOFF + dt * BVW + MARG + c0) - a_at
                    dst = AP(tensor=abf.tensor, offset=abf.offset + a_at,
                             ap=[list(abf.ap[0]), [gap, 2], [1, cn]])
                    nc.scalar.activation(dst, src, AF.Identity,
                                         bias=b1h[:, dt: dt + 1])
                # odd-tap-aligned copy of Bv (SBUF->SBUF, all dt fused)
                nc.vector.tensor_copy(
                    bvo4[:, :, MARG + 1 + c0: MARG + 1 + c0 + cn],
                    bv4[:, :, MARG + c0: MARG + c0 + cn])

            def emit_win(r, ci, c0, cn):
                hn = cn // 2
                for hb in (0, hn):
                    hc0 = c0 + hb
                    tmph = wpw.tile([P, DT * W9 * HCN], fp16, tag="tmp")
                    gh = wpw.tile([P, DT * W9 * HCN], fp16, tag="g")
                    tmp4 = tmph[:].rearrange("p (d k i) -> p d k i",
                                             d=DT, k=W9)
                    g4 = gh[:].rearrange("p (d k i) -> p d k i",
                                         d=DT, k=W9)
                    a4c = a4[:, :, hc0:hc0 + hn]
                    if FUSED_TAPS:
                        # tmp slot j holds tap k = 8-j (order-free: summed)
                        tb = tmph[:]
                        # evens j=0,2,4,6,8 <-> k=8,6,4,2,0 from Bv
                        out_e = strided4(tb, 0, [(W9 * HCN, DT),
                                                 (2 * HCN, 5), (1, hn)])
                        in_e = strided4(bv_flat, MARG + hc0 - 8,
                                        [(BVW, DT), (2, 5), (1, hn)])
                        a_b5 = a4c.unsqueeze(2).to_broadcast([P, DT, 5, hn])
                        nc.vector.tensor_tensor(out_e, a_b5, in_e, ALU.add)
                        # odds j=1,3,5,7 <-> k=7,5,3,1 from Bvo
                        out_o = strided4(tb, HCN, [(W9 * HCN, DT),
                                                   (2 * HCN, 4), (1, hn)])
                        in_o = strided4(bvo_flat, MARG + hc0 - 6,
                                        [(BVW, DT), (2, 4), (1, hn)])
                        a_b4 = a4c.unsqueeze(2).to_broadcast([P, DT, 4, hn])
                        nc.vector.tensor_tensor(out_o, a_b4, in_o, ALU.add)
                    else:
                        for k in range(W9):
                            if k % 2 == 0:
                                bsl = bv4[:, :, MARG + hc0 - k:
                                          MARG + hc0 - k + hn]
                            else:
                                bsl = bvo4[:, :, MARG + 1 + hc0 - k:
                                           MARG + 1 + hc0 - k + hn]
                            nc.vector.tensor_tensor(tmp4[:, :, k, 0:hn], a4c,
                                                    bsl, ALU.add)
                    nc.scalar.activation(g4[:, :, :, 0:hn],
                                         tmp4[:, :, :, 0:hn], AF.Gelu)
                    nc.vector.tensor_tensor(tmp4[:, :, 0:4, 0:hn],
                                            g4[:, :, 0:4, 0:hn],
                                            g4[:, :, 4:8, 0:hn], ALU.add)
                    nc.vector.tensor_tensor(tmp4[:, :, 0:2, 0:hn],
                                            tmp4[:, :, 0:2, 0:hn],
                                            tmp4[:, :, 2:4, 0:hn], ALU.add)
                    nc.vector.tensor_tensor(tmp4[:, :, 0, 0:hn],
                                            tmp4[:, :, 0, 0:hn],
                                            tmp4[:, :, 1, 0:hn], ALU.add)
                    nc.vector.tensor_tensor(d4(S, hc0, hn),
                                            tmp4[:, :, 0, 0:hn],
                                            g4[:, :, 8, 0:hn], ALU.add)

                # ---- edge fixup (chunk 0 only; no-op off sequence starts)
                if ci == 0:
                    ga8 = wpe.tile([P, DT * W], fp16, tag="ga8")
                    for dt in range(DT):
                        nc.scalar.activation(
                            ga8[:, dt * W: dt * W + W],
                            asl(dt, HALO, W),
                            AF.Gelu, bias=b1h[:, dt: dt + 1])
                    ga84 = ga8[:].rearrange("p (d w) -> p d w", d=DT)
                    s4e = d4(S, HALO, W)
                    ea_b = edge_a[:].unsqueeze(1).to_broadcast([P, DT, W])
                    es_b = edge_s[:].unsqueeze(1).to_broadcast([P, DT, W])
                    nc.vector.tensor_tensor(ga84, ga84, ea_b, ALU.mult)
                    nc.vector.tensor_tensor(s4e, s4e, ga84, ALU.subtract)
                    nc.vector.tensor_tensor(s4e, s4e, es_b, ALU.mult)

            def emit_r1(r, ci, c0, cn):
                hin = hbufs[r % 2]
                hout = hbufs[(r + 1) % 2]
                # ---- stage 4: U = u1t.T@h + wc.T@S ; G = gelu(U + ub1')
                for dt in range(DT):
                    pu = ps.tile([P, 512], f32, tag="pmm")
                    for kt in range(DT):
                        nc.tensor.matmul(pu[:, :cn], wtile(u1t, kt, dt),
                                         hsl(hin, kt, c0, cn),
                                         start=(kt == 0), stop=False)
                    for kt in range(DT):
                        nc.tensor.matmul(pu[:, :cn], wtile(wc, kt, dt),
                                         hsl(S, kt, c0, cn),
                                         start=False, stop=(kt == DT - 1))
                    nc.scalar.activation(hsl(G, dt, c0, cn), pu[:, :cn],
                                         AF.Gelu, bias=ub1[:, dt: dt + 1])
                # ---- stage 5: hout = (u2.T@G + ub2) + h  (residual in evac)
                for dt in range(DT):
                    pv = ps.tile([P, 512], f32, tag="pmm")
                    for kt in range(DT):
                        nc.tensor.matmul(pv[:, :cn], wtile(u2, kt, dt),
                                         hsl(G, kt, c0, cn),
                                         start=(kt == 0), stop=(kt == DT - 1))
                    nc.vector.scalar_tensor_tensor(
                        hsl(hout, dt, c0, cn), pv[:, :cn],
                        ub2[:, dt: dt + 1], hsl(hin, dt, c0, cn),
                        ALU.add, ALU.add)
                # x2 = hout^2
                x2t = wpx.tile([P, DT * CN], fp16, tag="x2")
                x24 = x2t[:].rearrange("p (d i) -> p d i", d=DT)[:, :, 0:cn]
                hout4 = d4(hout, c0, cn)
                nc.vector.tensor_tensor(x24, hout4, hout4, ALU.mult)
                return x2t

            def emit_r2(r, ci, c0, cn, x2t):
                hin = hbufs[r % 2]
                hout = hbufs[(r + 1) % 2]
                hout4 = d4(hout, c0, cn)
                x24 = x2t[:].rearrange("p (d i) -> p d i", d=DT)[:, :, 0:cn]
                # ---- LN stats via ones-matmul
                pr0 = psr.tile([P, 512], f32, tag="prow")
                pr1 = psr.tile([P, 512], f32, tag="prow")
                for kt in range(DT):
                    nc.tensor.matmul(pr0[:, :cn], ones_b[:],
                                     hsl(hout, kt, c0, cn),
                                     start=(kt == 0), stop=(kt == DT - 1))
                for kt in range(DT):
                    nc.tensor.matmul(pr1[:, :cn], ones_b[:],
                                     x2t[:, kt * CN: kt * CN + cn],
                                     start=(kt == 0), stop=(kt == DT - 1))
                mub = wps.tile([P, CN], fp16, tag="mub")
                qq = wps.tile([P, CN], fp16, tag="qq")
                vt = wps.tile([P, CN], fp16, tag="vt")
                y0 = wps.tile([P, CN], fp16, tag="y0")
                tt = wps.tile([P, CN], fp16, tag="tt")
                rsth = wps.tile([P, CN], fp16, tag="rsth")
                # mub = -pr0/D (ACT, parallel to the rsqrt chain)
                nc.scalar.activation(mub[:, :cn], pr0[:, :cn], AF.Copy,
                                     scale=-1.0 / D)
                # vt = D*var = pr1 - pr0^2/D; rst = rsqrt(var) computed as
                # sqrt(D)*rsqrt(vt) via a sqrt(D)-scaled seed magic and a
                # Newton step with 0.5/D in place of 0.5
                nc.vector.tensor_tensor(qq[:, :cn], mub[:, :cn],
                                        mub[:, :cn], ALU.mult)
                nc.vector.scalar_tensor_tensor(vt[:, :cn], qq[:, :cn],
                                               -float(D), pr1[:, :cn],
                                               ALU.mult, ALU.add)
                nc.vector.tensor_scalar(y0[:, :cn].bitcast(u16),
                                        vt[:, :cn].bitcast(u16), 1, None,
                                        ALU.logical_shift_right)
                nc.vector.tensor_tensor(y0[:, :cn].bitcast(u16),
                                        mg[:, :cn], y0[:, :cn].bitcast(u16),
                                        ALU.subtract)
                nc.vector.tensor_tensor(tt[:, :cn], y0[:, :cn], y0[:, :cn],
                                        ALU.mult)
                nc.vector.scalar_tensor_tensor(tt[:, :cn], tt[:, :cn],
                                               -0.5 / D, vt[:, :cn],
                                               ALU.mult, ALU.mult)
                nc.vector.scalar_tensor_tensor(rsth[:, :cn], tt[:, :cn], 1.5,
                                               y0[:, :cn], ALU.add, ALU.mult)
                # xm = hout - mu ; hout = xm * rst (final mult per dt so
                # next-round matmuls start as soon as each d-tile lands)
                mub_b = mub[:, :cn].unsqueeze(1).to_broadcast([P, DT, cn])
                nc.vector.tensor_tensor(x24, hout4, mub_b, ALU.add)
                for dt in range(DT):
                    nc.vector.tensor_tensor(
                        hsl(hout, dt, c0, cn),
                        x2t[:, dt * CN: dt * CN + cn], rsth[:, :cn],
                        ALU.mult)
                if ln_affine:
                    for dt in range(DT):
                        nc.vector.tensor_scalar(
                            hsl(hout, dt, c0, cn), hsl(hout, dt, c0, cn),
                            lng[:, dt: dt + 1], lnb[:, dt: dt + 1],
                            ALU.mult, ALU.add)

                # zero pad margin on sequence-start cores (chunk 0)
                if ci == 0 and r < N_ROUNDS - 1:
                    hm_b = hmask[:].unsqueeze(1).to_broadcast([P, DT, HALO])
                    nc.gpsimd.tensor_tensor(d4(hout, 0, HALO),
                                            d4(hout, 0, HALO), hm_b,
                                            ALU.mult)
                # stream the final round's output per chunk (overlaps drain)
                if r == N_ROUNDS - 1:
                    qs = [nc.sync, nc.gpsimd]
                    lo = max(c0, HALO)
                    hi = c0 + cn
                    for dt in range(DT):
                        qs[(dt + ci) % 2].dma_start(
                            out=out_d[dt, :, lo - HALO: hi - HALO],
                            in_=hsl(hout, dt, lo, hi - lo))

            # ---------- software-pipelined emission ----------
            # per iteration: S1(c) | R1(c-1) | W(c) | R2(c-2)
            pend1 = []   # chunks awaiting R1
            pend2 = []   # chunks awaiting R2 (carry x2t from R1)
            for r in range(N_ROUNDS):
                for ci, (c0, cn) in enumerate(CHUNKS_R[r]):
                    emit_s1(r, ci, c0, cn)
                    if pend1:
                        args = pend1.pop(0)
                        x2t = emit_r1(*args)
                        pend2.append((*args, x2t))
                    emit_win(r, ci, c0, cn)
                    if len(pend2) >= 2:
                        emit_r2(*pend2.pop(0))
                    pend1.append((r, ci, c0, cn))
            while pend1:
                args = pend1.pop(0)
                x2t = emit_r1(*args)
                pend2.append((*args, x2t))
                if pend2:
                    emit_r2(*pend2.pop(0))
            while pend2:
                emit_r2(*pend2.pop(0))

    nc.finalize()
    return nc


_NC_CACHE = {}


def _get_nc(ln_affine=False):
    key = ("nc", ln_affine)
    if key not in _NC_CACHE:
        _NC_CACHE[key] = build_nc(ln_affine)
    return _NC_CACHE[key]


def _prep_inputs(chunk_summaries, msg_w1, msg_b1, msg_w2, msg_b2,
                 upd_w1, upd_b1, upd_w2, upd_b2, ln_g, ln_b):
    h = np.asarray(chunk_summaries, np.float32)          # (B, N, D)
    w1 = np.asarray(msg_w1, np.float32)                  # (2D, D)
    w2 = np.asarray(msg_w2, np.float32)                  # (D, D)
    u1 = np.asarray(upd_w1, np.float32)
    u2 = np.asarray(upd_w2, np.float32)
    lng = np.asarray(ln_g, np.float32)
    lnb = np.asarray(ln_b, np.float32)
    ln_affine = not (np.all(lng == 1.0) and np.all(lnb == 0.0))

    # fold msg_b2 into the update-MLP hidden bias: agg@u1b + (b2@u1b + ub1)
    ub1f = (np.asarray(upd_b1, np.float64)
            + np.asarray(msg_b2, np.float64) @ np.asarray(u1[D:], np.float64)
            ).astype(np.float32)
    # fold the msg second layer into the update first layer:
    # agg@u1b = (S/9 @ w2)@u1b = S @ ((w2/9) @ u1b)
    wcomb = ((np.asarray(w2, np.float64) / 9.0)
             @ np.asarray(u1[D:], np.float64)).astype(np.float32)

    def pack_w(w):
        return np.ascontiguousarray(w.reshape(DT, P, D).astype(np.float16))

    def pack_b(b):
        return np.ascontiguousarray(np.asarray(b, np.float32).reshape(DT, P).T)

    common = {
        "w1t": pack_w(w1[:D]),
        "w1b": pack_w(w1[D:]),
        "u1t": pack_w(u1[:D]),
        "wc": pack_w(wcomb),
        "u2": pack_w(u2),
        "b1h": pack_b(np.asarray(msg_b1, np.float64) * 0.5),
        "ub1": pack_b(ub1f),
        "ub2": pack_b(upd_b2),
        "lng": pack_b(lng),
        "lnb": pack_b(lnb),
    }

    i8 = np.arange(W, dtype=np.float32)
    ea_edge = np.broadcast_to((W - i8), (P, W)).astype(np.float16)
    es_edge = np.broadcast_to((9.0 / (i8 + 1.0)), (P, W)).astype(np.float16)
    ea_mid = np.zeros((P, W), np.float16)
    es_mid = np.ones((P, W), np.float16)
    hm_edge = np.zeros((P, HALO), np.float16)
    hm_mid = np.ones((P, HALO), np.float16)

    in_maps = []
    for core in range(NCORES):
        b = core // 4
        q = core % 4
        n0 = q * NLOC
        if q == 0:
            loc = np.zeros((T, D), np.float32)
            loc[HALO:] = h[b, :NLOC]
            ea, es, hm = ea_edge, es_edge, hm_edge
        else:
            loc = h[b, n0 - HALO: n0 + NLOC]
            ea, es, hm = ea_mid, es_mid, hm_mid
        hloc = np.ascontiguousarray(
            loc.T.reshape(DT, P, T).astype(np.float16))
        m = dict(common)
        m["h_in"] = hloc
        m["edge_a"] = ea
        m["edge_s"] = es
        m["hmask"] = hm
        in_maps.append(m)
    return in_maps, ln_affine


def kernel(**inputs) -> np.ndarray:
    in_maps, ln_affine = _prep_inputs(**inputs)
    nc = _get_nc(ln_affine)
    res = run_bass_kernel_spmd(nc, in_maps, list(range(NCORES)))
    out = np.empty((B, N, D), np.float32)
    for core in range(NCORES):
        b = core // 4
        q = core % 4
        o = np.asarray(res.results[core]["out"]).astype(np.float32)
        out[b, q * NLOC:(q + 1) * NLOC] = o.reshape(D, NLOC).T
    return out


# revision 27
# speedup vs baseline: 1.0630x; 1.0182x over previous
"""Trainium2 Bass kernel for ChunkMessagePassing (gnn_message_passing).

Problem: B=2, N=4096, D=512, 3 rounds of causal windowed (W=8) message
passing. Per round:
    A = h @ w1_top ; Bv = h @ w1_bot + b1       (first MLP layer, factored)
    S[i] = sum_{k=0..8, valid} gelu(A[i] + Bv[i-k])
    hidden = gelu(h @ u1t + S @ Wc + ub1')      (Wc = (w2/9) @ u1b, host-folded;
                                                 b2 folded into ub1 on host)
    new_h = h + hidden @ u2 + ub2 ; h = LN(new_h)

Sharding: 8 cores = B(2) x N-quarters(4). Each core owns 1024 tokens plus a
24-token left halo (3 rounds x window 8) computed redundantly -> zero
cross-core communication. Sequence-start cores get a b1-filled margin plus a
data-driven edge fixup so all 8 cores run one SPMD program.

Layout: D on partitions (4 tiles of 128), tokens on the free axis. All
activations and weights fp16 (PSUM f32). Engine balance:
  - PE: 5 DxD matmuls/token/round (agg matmul folded into Wc on host) plus
    ones-matmul LN stats; software-pipelined emission (stage-1 of chunk c+1
    issues before stage-4/5 of chunk c, LN stats at lag 2) keeps the PE
    stream fed so the p-state ramp stays at full clock.
  - Pool (gpsimd): A/Bv PSUM evacuations + mub, off the ACT engine.
  - ACT: window gelu (bias-free; b1 rides the Bv evac), G gelu, x2 square.
  - DVE: fused window tap adds (2 instructions per half-chunk via strided
    APs with a k-reversed tmp layout), tap-sum tree, residual/LN chain.
  - LN: ones-matmul stats, rsqrt via f32 bit-trick + 1 Newton step; only
    gelu-set ACT functions are used so there is no table thrashing.
"""

import numpy as np

import concourse.bacc as bacc
import concourse.mybir as mybir
from concourse.tile import TileContext
from concourse.bass_utils import run_bass_kernel_spmd
from concourse.ap import AP

f32 = mybir.dt.float32
fp16 = mybir.dt.float16
u32 = mybir.dt.uint32
u16 = mybir.dt.uint16
AF = mybir.ActivationFunctionType
ALU = mybir.AluOpType

B, N, D = 2, 4096, 512
N_ROUNDS = 3
W = 8
W9 = W + 1
NCORES = 8
NLOC = N // 4            # tokens owned per core
HALO = N_ROUNDS * W      # 24
T = NLOC + HALO          # 1048 local tokens incl. halo
DT = 4                   # number of 128-partition d tiles
P = 128
MARG = 8                 # margin on the left of Bv buffers (holds b1)
BVW = MARG + T + 2
CN = 352                 # max chunk width
HCN = CN // 2            # window-stage half-chunk width
CHUNKS = [(0, 352), (352, 352), (704, 344)]
# per-round chunking: round 0 splits the first chunk (faster pipeline fill),
# every round splits the last chunk (shorter round-boundary / drain chain)
CHUNKS_R = [
    [(0, 176), (176, 176), (352, 352), (704, 172), (876, 172)],
    [(0, 352), (352, 352), (704, 172), (876, 172)],
    [(0, 352), (352, 352), (704, 172), (876, 172)],
]
# fp16 rsqrt seed magic, pre-scaled by sqrt(D): seeds y ~ sqrt(D)/sqrt(v)
MAGIC16 = 0x59BB + (9 << 9)

POOL_EVAC = False        # gpsimd cannot access PSUM on TRN2
FUSED_TAPS = True        # 2 strided tap instructions instead of 9


def build_nc(ln_affine=False):
    nc = bacc.Bacc("TRN2")

    # ---- DRAM I/O (per-core data supplied via in_maps) ----
    h_in = nc.dram_tensor("h_in", [DT, P, T], fp16, kind="ExternalInput")
    w1t_d = nc.dram_tensor("w1t", [DT, P, D], fp16, kind="ExternalInput")
    w1b_d = nc.dram_tensor("w1b", [DT, P, D], fp16, kind="ExternalInput")
    u1t_d = nc.dram_tensor("u1t", [DT, P, D], fp16, kind="ExternalInput")
    wc_d = nc.dram_tensor("wc", [DT, P, D], fp16, kind="ExternalInput")
    u2_d = nc.dram_tensor("u2", [DT, P, D], fp16, kind="ExternalInput")
    b1h_d = nc.dram_tensor("b1h", [P, DT], f32, kind="ExternalInput")
    ub1_d = nc.dram_tensor("ub1", [P, DT], f32, kind="ExternalInput")
    ub2_d = nc.dram_tensor("ub2", [P, DT], f32, kind="ExternalInput")
    lng_d = nc.dram_tensor("lng", [P, DT], f32, kind="ExternalInput")
    lnb_d = nc.dram_tensor("lnb", [P, DT], f32, kind="ExternalInput")
    ea_d = nc.dram_tensor("edge_a", [P, W], fp16, kind="ExternalInput")
    es_d = nc.dram_tensor("edge_s", [P, W], fp16, kind="ExternalInput")
    hm_d = nc.dram_tensor("hmask", [P, HALO], fp16, kind="ExternalInput")
    out_d = nc.dram_tensor("out", [DT, P, NLOC], fp16, kind="ExternalOutput")

    with nc.allow_low_precision("fp16 compute validated against reference"), \
            TileContext(nc) as tc:
        with (
            tc.tile_pool(name="const", bufs=1) as cp,
            tc.tile_pool(name="acts", bufs=1) as ap,
            tc.tile_pool(name="win", bufs=3) as wpw,
            tc.tile_pool(name="x2p", bufs=2) as wpx,
            tc.tile_pool(name="lns", bufs=8) as wps,
            tc.tile_pool(name="edg", bufs=2) as wpe,
            tc.tile_pool(name="psab", bufs=2, space="PSUM") as psab,
            tc.tile_pool(name="ps", bufs=2, space="PSUM") as ps,
            tc.tile_pool(name="psr", bufs=2, space="PSUM") as psr,
        ):
            # ---- constants into SBUF ----
            w1t = cp.tile([P, DT * D], fp16, tag="w1t")
            w1b = cp.tile([P, DT * D], fp16, tag="w1b")
            u1t = cp.tile([P, DT * D], fp16, tag="u1t")
            wc = cp.tile([P, DT * D], fp16, tag="wc")
            u2 = cp.tile([P, DT * D], fp16, tag="u2")
            b1h = cp.tile([P, DT], f32, tag="b1h")
            ub1 = cp.tile([P, DT], f32, tag="ub1")
            ub2 = cp.tile([P, DT], f32, tag="ub2")
            lng = cp.tile([P, DT], f32, tag="lng")
            lnb = cp.tile([P, DT], f32, tag="lnb")
            edge_a = cp.tile([P, W], fp16, tag="edge_a")
            edge_s = cp.tile([P, W], fp16, tag="edge_s")
            hmask = cp.tile([P, HALO], fp16, tag="hmask")
            # smalls first on the scalar queue (b1h gates the margins), then
            # nothing else on scalar so chunk-0 evacuations start promptly
            for t_sb, t_d in ((b1h, b1h_d), (ub1, ub1_d), (ub2, ub2_d),
                              (lng, lng_d), (lnb, lnb_d), (edge_a, ea_d),
                              (edge_s, es_d), (hmask, hm_d)):
                nc.scalar.dma_start(out=t_sb[:], in_=t_d[:])
            # weight order on sync matches first-use order; the h chunk for
            # tokens 352..704 rides between so stage-1 of chunk 2 is fed
            for t_sb, t_d in ((w1t, w1t_d), (w1b, w1b_d)):
                for kt in range(DT):
                    nc.sync.dma_start(out=t_sb[:, kt * D: (kt + 1) * D],
                                      in_=t_d[kt])

            ones_b = cp.tile([P, P], fp16, tag="ones_b")   # stats lhsT
            nc.vector.memset(ones_b[:], 1.0)
            mg = cp.tile([P, CN], u16, tag="mg")           # rsqrt seed magic
            nc.vector.memset(mg[:], MAGIC16)
            czero = cp.tile([P, 1], f32, tag="czero")
            nc.vector.memset(czero[:], 0.0)
            nc.const_aps.aps[(f32, 0.0)] = czero[:]
            # dummy gelu so the preamble loads the gelu act table once,
            # instead of a mid-stream 1.3us ACT_TABLE_LOAD stall
            warm = cp.tile([P, 1], fp16, tag="warm")
            nc.scalar.activation(warm[:], czero[:], AF.Gelu)

            # ---- activations (persistent, reused across rounds) ----
            h0 = ap.tile([P, DT * T], fp16, tag="h0")
            h1 = ap.tile([P, DT * T], fp16, tag="h1")
            # A and Bv share one tensor so the paired PSUM evacuation can
            # write both with a single strided instruction
            ABt = ap.tile([P, DT * T + DT * BVW], fp16, tag="ABt")
            AOFF = 0
            BVOFF = DT * T
            G = ap.tile([P, DT * T], fp16, tag="G")
            Bvo = ap.tile([P, DT * BVW], fp16, tag="Bvo")
            S = ap.tile([P, DT * T], fp16, tag="S")

            # round-1 input: chunks 0,2 on gpsimd (early), chunk 1 on sync
            # (rides between the w1 and u1 weight groups)
            for li, (c0, cn) in enumerate(CHUNKS):
                q = nc.gpsimd if li != 1 else nc.sync
                for dt in range(DT):
                    q.dma_start(out=h0[:, dt * T + c0: dt * T + c0 + cn],
                                in_=h_in[dt, :, c0: c0 + cn])
            for t_sb, t_d in ((u1t, u1t_d), (wc, wc_d), (u2, u2_d)):
                for kt in range(DT):
                    nc.sync.dma_start(out=t_sb[:, kt * D: (kt + 1) * D],
                                      in_=t_d[kt])

            # margins: b1/2 per dt (both A and Bv carry half of b1, so
            # invalid window taps see gelu(A + b1/2 + b1/2) = gelu(A + b1))
            for dt in range(DT):
                nc.vector.tensor_copy(
                    ABt[:, BVOFF + dt * BVW: BVOFF + dt * BVW + MARG],
                    b1h[:, dt: dt + 1].to_broadcast([P, MARG]))
                nc.vector.tensor_copy(
                    Bvo[:, dt * BVW: dt * BVW + MARG + 1],
                    b1h[:, dt: dt + 1].to_broadcast([P, MARG + 1]))

            def hsl(h, dt, c0, n):
                return h[:, dt * T + c0: dt * T + c0 + n]

            def asl(dt, c0, n):
                return ABt[:, AOFF + dt * T + c0: AOFF + dt * T + c0 + n]

            def wtile(w, kt, dt):
                return w[:, kt * D + dt * P: kt * D + dt * P + P]

            def d4(tile, c0, cn):       # [P, 4dt, cn] view of a [P, DT*T] tile
                return tile[:].rearrange("p (d t) -> p d t", d=DT)[:, :, c0:c0 + cn]

            a_part = ABt[:][:, AOFF: AOFF + DT * T]
            a4 = a_part.rearrange("p (d t) -> p d t", d=DT)
            bv_flat = ABt[:][:, BVOFF: BVOFF + DT * BVW]
            bv4 = bv_flat.rearrange("p (d v) -> p d v", d=DT)
            bvo4 = Bvo[:].rearrange("p (d v) -> p d v", d=DT)
            bvo_flat = Bvo[:]

            def strided4(base, off, dims):
                # raw AP: dims = [(stride, num), ...] free dims (<=3)
                return AP(tensor=base.tensor, offset=base.offset + off,
                          ap=[list(base.ap[0])] + [list(d) for d in dims])

            hbufs = [h0, h1]

            # ---------- per-chunk emitters ----------
            def emit_s1(r, ci, c0, cn):
                hin = hbufs[r % 2]
                abf = ABt[:]
                for dt in range(DT):
                    pab = psab.tile([P, 1024], f32, tag="pab")
                    for kt in range(DT):
                        nc.tensor.matmul(pab[:, :cn], wtile(w1t, kt, dt),
                                         hsl(hin, kt, c0, cn),
                                         start=(kt == 0), stop=(kt == DT - 1))
                    for kt in range(DT):
                        nc.tensor.matmul(pab[:, 512: 512 + cn],
                                         wtile(w1b, kt, dt),
                                         hsl(hin, kt, c0, cn),
                                         start=(kt == 0), stop=(kt == DT - 1))
                    # paired evacuation: A' = A + b1/2, Bv' = Bv + b1/2 in
                    # one strided ACT instruction across the 2 PSUM banks
                    psrc = pab[:]
                    src = AP(tensor=psrc.tensor, offset=psrc.offset,
                             ap=[list(psrc.ap[0]), [512, 2], [1, cn]])
                    a_at = AOFF + dt * T + c0
                    gap = (BVOFF + dt * BVW + MARG + c0) - a_at
                    dst = AP(tensor=abf.tensor, offset=abf.offset + a_at,
                             ap=[list(abf.ap[0]), [gap, 2], [1, cn]])
                    nc.scalar.activation(dst, src, AF.Identity,
                                         bias=b1h[:, dt: dt + 1])
                # odd-tap-aligned copy of Bv (SBUF->SBUF, all dt fused)
                nc.vector.tensor_copy(
                    bvo4[:, :, MARG + 1 + c0: MARG + 1 + c0 + cn],
                    bv4[:, :, MARG + c0: MARG + c0 + cn])

            def emit_win(r, ci, c0, cn):
                hn = cn // 2
                for hb in (0, hn):
                    hc0 = c0 + hb
                    tmph = wpw.tile([P, DT * W9 * HCN], fp16, tag="tmp")
                    gh = wpw.tile([P, DT * W9 * HCN], fp16, tag="g")
                    tmp4 = tmph[:].rearrange("p (d k i) -> p d k i",
                                             d=DT, k=W9)
                    g4 = gh[:].rearrange("p (d k i) -> p d k i",
                                         d=DT, k=W9)
                    a4c = a4[:, :, hc0:hc0 + hn]
                    if FUSED_TAPS:
                        # tmp slot j holds tap k = 8-j (order-free: summed)
                        tb = tmph[:]
                        # evens j=0,2,4,6,8 <-> k=8,6,4,2,0 from Bv
                        out_e = strided4(tb, 0, [(W9 * HCN, DT),
                                                 (2 * HCN, 5), (1, hn)])
                        in_e = strided4(bv_flat, MARG + hc0 - 8,
                                        [(BVW, DT), (2, 5), (1, hn)])
                        a_b5 = a4c.unsqueeze(2).to_broadcast([P, DT, 5, hn])
                        nc.vector.tensor_tensor(out_e, a_b5, in_e, ALU.add)
                        # odds j=1,3,5,7 <-> k=7,5,3,1 from Bvo
                        out_o = strided4(tb, HCN, [(W9 * HCN, DT),
                                                   (2 * HCN, 4), (1, hn)])
                        in_o = strided4(bvo_flat, MARG + hc0 - 6,
                                        [(BVW, DT), (2, 4), (1, hn)])
                        a_b4 = a4c.unsqueeze(2).to_broadcast([P, DT, 4, hn])
                        nc.vector.tensor_tensor(out_o, a_b4, in_o, ALU.add)
                    else:
                        for k in range(W9):
                            if k % 2 == 0:
                                bsl = bv4[:, :, MARG + hc0 - k:
                                          MARG + hc0 - k + hn]
                            else:
                                bsl = bvo4[:, :, MARG + 1 + hc0 - k:
                                           MARG + 1 + hc0 - k + hn]
                            nc.vector.tensor_tensor(tmp4[:, :, k, 0:hn], a4c,
                                                    bsl, ALU.add)
                    nc.scalar.activation(g4[:, :, :, 0:hn],
                                         tmp4[:, :, :, 0:hn], AF.Gelu)
                    nc.vector.tensor_tensor(tmp4[:, :, 0:4, 0:hn],
                                            g4[:, :, 0:4, 0:hn],
                                            g4[:, :, 4:8, 0:hn], ALU.add)
                    nc.vector.tensor_tensor(tmp4[:, :, 0:2, 0:hn],
                                            tmp4[:, :, 0:2, 0:hn],
                                            tmp4[:, :, 2:4, 0:hn], ALU.add)
                    nc.vector.tensor_tensor(tmp4[:, :, 0, 0:hn],
                                            tmp4[:, :, 0, 0:hn],
                                            tmp4[:, :, 1, 0:hn], ALU.add)
                    nc.vector.tensor_tensor(d4(S, hc0, hn),
                                            tmp4[:, :, 0, 0:hn],
                                            g4[:, :, 8, 0:hn], ALU.add)

                # ---- edge fixup (chunk 0 only; no-op off sequence starts)
                if ci == 0:
                    ga8 = wpe.tile([P, DT * W], fp16, tag="ga8")
                    for dt in range(DT):
                        nc.scalar.activation(
                            ga8[:, dt * W: dt * W + W],
                            asl(dt, HALO, W),
                            AF.Gelu, bias=b1h[:, dt: dt + 1])
                    ga84 = ga8[:].rearrange("p (d w) -> p d w", d=DT)
                    s4e = d4(S, HALO, W)
                    ea_b = edge_a[:].unsqueeze(1).to_broadcast([P, DT, W])
                    es_b = edge_s[:].unsqueeze(1).to_broadcast([P, DT, W])
                    nc.vector.tensor_tensor(ga84, ga84, ea_b, ALU.mult)
                    nc.vector.tensor_tensor(s4e, s4e, ga84, ALU.subtract)
                    nc.vector.tensor_tensor(s4e, s4e, es_b, ALU.mult)

            def emit_r1(r, ci, c0, cn):
                hin = hbufs[r % 2]
                hout = hbufs[(r + 1) % 2]
                # ---- stage 4: U = u1t.T@h + wc.T@S ; G = gelu(U + ub1')
                for dt in range(DT):
                    pu = ps.tile([P, 512], f32, tag="pmm")
                    for kt in range(DT):
                        nc.tensor.matmul(pu[:, :cn], wtile(u1t, kt, dt),
                                         hsl(hin, kt, c0, cn),
                                         start=(kt == 0), stop=False)
                    for kt in range(DT):
                        nc.tensor.matmul(pu[:, :cn], wtile(wc, kt, dt),
                                         hsl(S, kt, c0, cn),
                                         start=False, stop=(kt == DT - 1))
                    nc.scalar.activation(hsl(G, dt, c0, cn), pu[:, :cn],
                                         AF.Gelu, bias=ub1[:, dt: dt + 1])
                # ---- stage 5: hout = (u2.T@G + ub2) + h  (residual in evac)
                for dt in range(DT):
                    pv = ps.tile([P, 512], f32, tag="pmm")
                    for kt in range(DT):
                        nc.tensor.matmul(pv[:, :cn], wtile(u2, kt, dt),
                                         hsl(G, kt, c0, cn),
                                         start=(kt == 0), stop=(kt == DT - 1))
                    nc.vector.scalar_tensor_tensor(
                        hsl(hout, dt, c0, cn), pv[:, :cn],
                        ub2[:, dt: dt + 1], hsl(hin, dt, c0, cn),
                        ALU.add, ALU.add)
                # x2 = hout^2
                x2t = wpx.tile([P, DT * CN], fp16, tag="x2")
                x24 = x2t[:].rearrange("p (d i) -> p d i", d=DT)[:, :, 0:cn]
                hout4 = d4(hout, c0, cn)
                nc.vector.tensor_tensor(x24, hout4, hout4, ALU.mult)
                return x2t

            def emit_r2(r, ci, c0, cn, x2t):
                hin = hbufs[r % 2]
                hout = hbufs[(r + 1) % 2]
                hout4 = d4(hout, c0, cn)
                x24 = x2t[:].rearrange("p (d i) -> p d i", d=DT)[:, :, 0:cn]
                # ---- LN stats via ones-matmul
                pr0 = psr.tile([P, 512], f32, tag="prow")
                pr1 = psr.tile([P, 512], f32, tag="prow")
                for kt in range(DT):
                    nc.tensor.matmul(pr0[:, :cn], ones_b[:],
                                     hsl(hout, kt, c0, cn),
                                     start=(kt == 0), stop=(kt == DT - 1))
                for kt in range(DT):
                    nc.tensor.matmul(pr1[:, :cn], ones_b[:],
                                     x2t[:, kt * CN: kt * CN + cn],
                                     start=(kt == 0), stop=(kt == DT - 1))
                mub = wps.tile([P, CN], fp16, tag="mub")
                qq = wps.tile([P, CN], fp16, tag="qq")
                vt = wps.tile([P, CN], fp16, tag="vt")
                y0 = wps.tile([P, CN], fp16, tag="y0")
                tt = wps.tile([P, CN], fp16, tag="tt")
                rsth = wps.tile([P, CN], fp16, tag="rsth")
                # mub = -pr0/D (ACT, parallel to the rsqrt chain)
                nc.scalar.activation(mub[:, :cn], pr0[:, :cn], AF.Copy,
                                     scale=-1.0 / D)
                # vt = D*var = pr1 - pr0^2/D; rst = rsqrt(var) computed as
                # sqrt(D)*rsqrt(vt) via a sqrt(D)-scaled seed magic and a
                # Newton step with 0.5/D in place of 0.5
                nc.vector.tensor_tensor(qq[:, :cn], mub[:, :cn],
                                        mub[:, :cn], ALU.mult)
                nc.vector.scalar_tensor_tensor(vt[:, :cn], qq[:, :cn],
                                               -float(D), pr1[:, :cn],
                                               ALU.mult, ALU.add)
                nc.vector.tensor_scalar(y0[:, :cn].bitcast(u16),
                                        vt[:, :cn].bitcast(u16), 1, None,
                                        ALU.logical_shift_right)
                nc.vector.tensor_tensor(y0[:, :cn].bitcast(u16),
                                        mg[:, :cn], y0[:, :cn].bitcast(u16),
                                        ALU.subtract)
                nc.vector.tensor_tensor(tt[:, :cn], y0[:, :cn], y0[:, :cn],
                                        ALU.mult)
                nc.vector.scalar_tensor_tensor(tt[:, :cn], tt[:, :cn],
                                               -0.5 / D, vt[:, :cn],
                                               ALU.mult, ALU.mult)
                nc.vector.scalar_tensor_tensor(rsth[:, :cn], tt[:, :cn], 1.5,
                                               y0[:, :cn], ALU.add, ALU.mult)
                # xm = hout - mu ; hout = xm * rst (final mult per dt so
                # next-round matmuls start as soon as each d-tile lands)
                mub_b = mub[:, :cn].unsqueeze(1).to_broadcast([P, DT, cn])
                nc.vector.tensor_tensor(x24, hout4, mub_b, ALU.add)
                for dt in range(DT):
                    nc.vector.tensor_tensor(
                        hsl(hout, dt, c0, cn),
                        x2t[:, dt * CN: dt * CN + cn], rsth[:, :cn],
                        ALU.mult)
                if ln_affine:
                    for dt in range(DT):
                        nc.vector.tensor_scalar(
                            hsl(hout, dt, c0, cn), hsl(hout, dt, c0, cn),
                            lng[:, dt: dt + 1], lnb[:, dt: dt + 1],
                            ALU.mult, ALU.add)

                # zero pad margin on sequence-start cores (chunk 0)
                if ci == 0 and r < N_ROUNDS - 1:
                    hm_b = hmask[:].unsqueeze(1).to_broadcast([P, DT, HALO])
                    nc.gpsimd.tensor_tensor(d4(hout, 0, HALO),
                                            d4(hout, 0, HALO), hm_b,
                                            ALU.mult)
                # stream the final round's output per chunk (overlaps drain)
                if r == N_ROUNDS - 1:
                    qs = [nc.sync, nc.gpsimd]
                    lo = max(c0, HALO)
                    hi = c0 + cn
                    for dt in range(DT):
                        qs[(dt + ci) % 2].dma_start(
                            out=out_d[dt, :, lo - HALO: hi - HALO],
                            in_=hsl(hout, dt, lo, hi - lo))

            # ---------- software-pipelined emission ----------
            # per iteration: S1(c) | R1(c-1) | W(c) | R2(c-2)
            pend1 = []   # chunks awaiting R1
            pend2 = []   # chunks awaiting R2 (carry x2t from R1)
            for r in range(N_ROUNDS):
                for ci, (c0, cn) in enumerate(CHUNKS_R[r]):
                    emit_s1(r, ci, c0, cn)
                    if pend1:
                        args = pend1.pop(0)
                        x2t = emit_r1(*args)
                        pend2.append((*args, x2t))
                    emit_win(r, ci, c0, cn)
                    if len(pend2) >= 2:
                        emit_r2(*pend2.pop(0))
                    pend1.append((r, ci, c0, cn))
            while pend1:
                args = pend1.pop(0)
                x2t = emit_r1(*args)
                pend2.append((*args, x2t))
                if pend2:
                    emit_r2(*pend2.pop(0))
            while pend2:
                emit_r2(*pend2.pop(0))

    nc.finalize()
    return nc


_NC_CACHE = {}


def _get_nc(ln_affine=False):
    key = ("nc", ln_affine)
    if key not in _NC_CACHE:
        _NC_CACHE[key] = build_nc(ln_affine)
    return _NC_CACHE[key]


def _prep_inputs(chunk_summaries, msg_w1, msg_b1, msg_w2, msg_b2,
                 upd_w1, upd_b1, upd_w2, upd_b2, ln_g, ln_b):
    h = np.asarray(chunk_summaries, np.float32)          # (B, N, D)
    w1 = np.asarray(msg_w1, np.float32)                  # (2D, D)
    w2 = np.asarray(msg_w2, np.float32)                  # (D, D)
    u1 = np.asarray(upd_w1, np.float32)
    u2 = np.asarray(upd_w2, np.float32)
    lng = np.asarray(ln_g, np.float32)
    lnb = np.asarray(ln_b, np.float32)
    ln_affine = not (np.all(lng == 1.0) and np.all(lnb == 0.0))

    # fold msg_b2 into the update-MLP hidden bias: agg@u1b + (b2@u1b + ub1)
    ub1f = (np.asarray(upd_b1, np.float64)
            + np.asarray(msg_b2, np.float64) @ np.asarray(u1[D:], np.float64)
            ).astype(np.float32)
    # fold the msg second layer into the update first layer:
    # agg@u1b = (S/9 @ w2)@u1b = S @ ((w2/9) @ u1b)
    wcomb = ((np.asarray(w2, np.float64) / 9.0)
             @ np.asarray(u1[D:], np.float64)).astype(np.float32)

    def pack_w(w):
        return np.ascontiguousarray(w.reshape(DT, P, D).astype(np.float16))

    def pack_b(b):
        return np.ascontiguousarray(np.asarray(b, np.float32).reshape(DT, P).T)

    common = {
        "w1t": pack_w(w1[:D]),
        "w1b": pack_w(w1[D:]),
        "u1t": pack_w(u1[:D]),
        "wc": pack_w(wcomb),
        "u2": pack_w(u2),
        "b1h": pack_b(np.asarray(msg_b1, np.float64) * 0.5),
        "ub1": pack_b(ub1f),
        "ub2": pack_b(upd_b2),
        "lng": pack_b(lng),
        "lnb": pack_b(lnb),
    }

    i8 = np.arange(W, dtype=np.float32)
    ea_edge = np.broadcast_to((W - i8), (P, W)).astype(np.float16)
    es_edge = np.broadcast_to((9.0 / (i8 + 1.0)), (P, W)).astype(np.float16)
    ea_mid = np.zeros((P, W), np.float16)
    es_mid = np.ones((P, W), np.float16)
    hm_edge = np.zeros((P, HALO), np.float16)
    hm_mid = np.ones((P, HALO), np.float16)

    in_maps = []
    for core in range(NCORES):
        b = core // 4
        q = core % 4
        n0 = q * NLOC
        if q == 0:
            loc = np.zeros((T, D), np.float32)
            loc[HALO:] = h[b, :NLOC]
            ea, es, hm = ea_edge, es_edge, hm_edge
        else:
            loc = h[b, n0 - HALO: n0 + NLOC]
            ea, es, hm = ea_mid, es_mid, hm_mid
        hloc = np.ascontiguousarray(
            loc.T.reshape(DT, P, T).astype(np.float16))
        m = dict(common)
        m["h_in"] = hloc
        m["edge_a"] = ea
        m["edge_s"] = es
        m["hmask"] = hm
        in_maps.append(m)
    return in_maps, ln_affine


def kernel(**inputs) -> np.ndarray:
    in_maps, ln_affine = _prep_inputs(**inputs)
    nc = _get_nc(ln_affine)
    res = run_bass_kernel_spmd(nc, in_maps, list(range(NCORES)))
    out = np.empty((B, N, D), np.float32)
    for core in range(NCORES):
        b = core // 4
        q = core % 4
        o = np.asarray(res.results[core]["out"]).astype(np.float32)
        out[b, q * NLOC:(q + 1) * NLOC] = o.reshape(D, NLOC).T
    return out
